# revision 1
# baseline (speedup 1.0000x reference)
"""GroupedQueryAttention (B=1, T=2048, D=4096, 32 q-heads / 8 kv-heads, hd=128)
on 8 trn2 NeuronCores.

Sharding: kv-head parallel — core c owns kv head c and its 4 query heads.
Each core: QKV projections (x.T streamed once), rope, causal attention in
transposed [k, q] score layout (softmax without max-subtraction: fp32 exp
can't overflow at these score magnitudes), AllGather of y.T, then a
column-parallel wo matmul so no AllReduce is needed.
"""
import sys

sys.path.insert(0, "/opt/trn_rl_repo")

import numpy as np

import concourse.bacc as bacc
import concourse.tile as tile
from concourse import mybir
from concourse.bass_utils import run_bass_kernel_spmd
from concourse.masks import make_identity

N_CORES = 8
T = 2048
DIM = 4096
HD = 128
NH = 32
NKV = 8
NREP = NH // NKV  # 4 query heads per core
NCHUNK = T // 512  # 4 chunks of 512 along T
NKT = DIM // 128  # 32 contraction tiles for the projections
NTT = T // 128  # 16 row tiles for the wo matmul
F32 = mybir.dt.float32
SCALE = 1.0 / float(np.sqrt(HD))

_cached = {}


import os

DEBUG = bool(int(os.environ.get("GQA_DEBUG", "0")))
MAXPHASE = int(os.environ.get("GQA_MAXPHASE", "4"))


def _build_kernel():
    if "nc" in _cached:
        return _cached["nc"]

    nc = bacc.Bacc("TRN2", target_bir_lowering=False)

    xT = nc.dram_tensor("xT", [DIM, T], F32, kind="ExternalInput")
    cos2 = nc.dram_tensor("cos2", [128, T], F32, kind="ExternalInput")
    sin2 = nc.dram_tensor("sin2", [128, T], F32, kind="ExternalInput")
    masks = nc.dram_tensor("masks", [128, 4 * 512], F32, kind="ExternalInput")
    wqT = nc.dram_tensor("wqT", [DIM, NREP * HD], F32, kind="ExternalInput")
    wkT = nc.dram_tensor("wkT", [DIM, HD], F32, kind="ExternalInput")
    wvT = nc.dram_tensor("wvT", [DIM, HD], F32, kind="ExternalInput")
    woT = nc.dram_tensor("woT", [DIM, NREP * HD], F32, kind="ExternalInput")
    out = nc.dram_tensor("out", [T, NREP * HD], F32, kind="ExternalOutput")

    if DEBUG:
        dbg_q = nc.dram_tensor("dbg_q", [128, NREP * T], F32, kind="ExternalOutput")
        dbg_k = nc.dram_tensor("dbg_k", [128, T], F32, kind="ExternalOutput")
        dbg_v = nc.dram_tensor("dbg_v", [128, NTT * HD], F32, kind="ExternalOutput")
        dbg_y = nc.dram_tensor("dbg_y", [NREP * HD, T], F32, kind="ExternalOutput")

    y_in = nc.dram_tensor("y_in", [NREP * HD, T], F32, kind="Internal")
    y_all = nc.dram_tensor(
        "y_all", [DIM, T], F32, kind="Internal", addr_space="Shared"
    )

    with tile.TileContext(nc) as tc:
        with (
            tc.tile_pool(name="consts", bufs=1) as consts,
            tc.tile_pool(name="weights", bufs=1) as weights,
            tc.tile_pool(name="acts", bufs=1) as acts,
            tc.tile_pool(name="stream", bufs=3) as stream,
            tc.tile_pool(name="work", bufs=2) as work,
            tc.tile_pool(name="expp", bufs=4) as expp,
            tc.tile_pool(name="outp", bufs=3) as outp,
            tc.tile_pool(name="psum", bufs=8, space="PSUM") as psum,
        ):
            # ---------- constants ----------
            cos_sb = consts.tile([128, T], F32, tag="cos")
            nc.sync.dma_start(out=cos_sb, in_=cos2[:, :])
            sin_sb = consts.tile([128, T], F32, tag="sin")
            nc.sync.dma_start(out=sin_sb, in_=sin2[:, :])
            mask_sb = consts.tile([128, 4 * 512], F32, tag="mask")
            nc.sync.dma_start(out=mask_sb, in_=masks[:, :])
            ones_col = consts.tile([128, 1], F32, tag="onesc")
            nc.vector.memset(ones_col, 1.0)
            ones_row = consts.tile([1, 128], F32, tag="onesr")
            nc.vector.memset(ones_row, 1.0)
            ident = consts.tile([128, 128], F32, tag="ident")
            make_identity(nc, ident)

            # ---------- resident weights (wq now; wo reuses the slot later) ----------
            wq_sb = weights.tile([128, NKT, NREP * HD], F32, tag="wbig")
            nc.sync.dma_start(
                out=wq_sb, in_=wqT.rearrange("(n p) m -> p n m", p=128)
            )

            # activations that live through the attention phase
            qT_sb = acts.tile([128, NREP, T], F32, tag="qt")
            kT_sb = acts.tile([128, T], F32, tag="kt")
            vkd_sb = acts.tile([128, NTT, HD], F32, tag="vkd")

            # ---------- phase 1: QKV projections + rope, chunk by chunk ----------
            for qc in range(NCHUNK):
                cs = slice(512 * qc, 512 * (qc + 1))
                q_ps = [
                    psum.tile([128, 512], F32, tag="bank", name=f"qps{qc}_{h}")
                    for h in range(NREP)
                ]
                k_ps = psum.tile([128, 512], F32, tag="bank")
                v_ps = psum.tile([128, 512], F32, tag="bank")
                for kt in range(NKT):
                    xt = stream.tile([128, 512], F32, tag="xt")
                    nc.sync.dma_start(
                        out=xt, in_=xT[128 * kt:128 * (kt + 1), cs]
                    )
                    wk_t = stream.tile([128, HD], F32, tag="wk")
                    nc.sync.dma_start(
                        out=wk_t, in_=wkT[128 * kt:128 * (kt + 1), :]
                    )
                    wv_t = stream.tile([128, HD], F32, tag="wv")
                    nc.sync.dma_start(
                        out=wv_t, in_=wvT[128 * kt:128 * (kt + 1), :]
                    )
                    st = kt == 0
                    sp = kt == NKT - 1
                    for h in range(NREP):
                        nc.tensor.matmul(
                            q_ps[h],
                            lhsT=wq_sb[:, kt, 128 * h:128 * (h + 1)],
                            rhs=xt,
                            start=st,
                            stop=sp,
                        )
                    nc.tensor.matmul(k_ps, lhsT=wk_t, rhs=xt, start=st, stop=sp)
                    nc.tensor.matmul(v_ps, lhsT=wv_t, rhs=xt, start=st, stop=sp)

                # v computed in [hd, T] layout; transpose 128x128 blocks to [k, hd]
                v_sb = work.tile([128, 512], F32, tag="vsb")
                nc.scalar.copy(v_sb, v_ps)
                for s in range(4):
                    vt_ps = psum.tile([128, 128], F32, tag="bank", name=f"vt{qc}_{s}")
                    nc.tensor.transpose(vt_ps, v_sb[:, 128 * s:128 * (s + 1)], ident)
                    nc.scalar.copy(vkd_sb[:, 4 * qc + s, :], vt_ps)

                # rope for the 4 q heads and k
                for h in range(NREP + 1):
                    p = q_ps[h] if h < NREP else k_ps
                    dst = qT_sb[:, h, cs] if h < NREP else kT_sb[:, cs]
                    sw = work.tile([128, 512], F32, tag="sw")
                    nc.scalar.copy(sw[0:64, :], p[64:128, :])
                    nc.scalar.copy(sw[64:128, :], p[0:64, :])
                    # dst = p * cos + sw * (+-sin)
                    nc.vector.tensor_mul(dst, p, cos_sb[:, cs])
                    nc.vector.tensor_mul(sw, sw, sin_sb[:, cs])
                    nc.vector.tensor_add(dst, dst, sw)

            # ---------- phase 2: causal attention, transposed score layout ----------
            for h in range(NREP if MAXPHASE >= 2 else 0):
                for qc in range(NCHUNK):
                    cs = slice(512 * qc, 512 * (qc + 1))
                    nkt = 4 * qc + 4  # causal: k tiles 0 .. 4*qc+3
                    yT_ps = psum.tile([128, 512], F32, tag="bank")
                    l_acc = work.tile([128, 512], F32, tag="lacc")
                    nc.vector.memset(l_acc, 0.0)
                    for kt in range(nkt):
                        sT_ps = psum.tile([128, 512], F32, tag="bank")
                        nc.tensor.matmul(
                            sT_ps,
                            lhsT=kT_sb[:, 128 * kt:128 * (kt + 1)],
                            rhs=qT_sb[:, h, cs],
                            start=True,
                            stop=True,
                        )
                        e_sb = expp.tile([128, 512], F32, tag="exp")
                        nc.scalar.activation(
                            e_sb, sT_ps, mybir.ActivationFunctionType.Exp,
                            scale=SCALE,
                        )
                        d = kt - 4 * qc
                        if d >= 0:  # diagonal block: zero the k > q half
                            nc.vector.tensor_mul(
                                e_sb, e_sb, mask_sb[:, 512 * d:512 * (d + 1)]
                            )
                        nc.vector.tensor_add(l_acc, l_acc, e_sb)
                        nc.tensor.matmul(
                            yT_ps,
                            lhsT=vkd_sb[:, kt, :],
                            rhs=e_sb,
                            start=(kt == 0),
                            stop=(kt == nkt - 1),
                        )
                    # softmax denominator -> reciprocal -> broadcast to 128 parts
                    l_ps = psum.tile([128, 512], F32, tag="bank")
                    nc.tensor.matmul(
                        l_ps[0:1, :], lhsT=ones_col[:, 0:1], rhs=l_acc,
                        start=True, stop=True,
                    )
                    recip = work.tile([1, 512], F32, tag="recip")
                    nc.vector.reciprocal(recip, l_ps[0:1, :])
                    bc_ps = psum.tile([128, 512], F32, tag="bank")
                    nc.tensor.matmul(
                        bc_ps, lhsT=ones_row[0:1, :], rhs=recip[0:1, :],
                        start=True, stop=True,
                    )
                    bc_sb = work.tile([128, 512], F32, tag="bc")
                    nc.scalar.copy(bc_sb, bc_ps)
                    yn_sb = work.tile([128, 512], F32, tag="yn")
                    nc.vector.tensor_mul(yn_sb, yT_ps, bc_sb)
                    nc.sync.dma_start(
                        out=y_in[128 * h:128 * (h + 1), cs], in_=yn_sb
                    )

            if DEBUG:
                nc.sync.dma_start(
                    out=dbg_q.rearrange("p (n m) -> p n m", n=NREP), in_=qT_sb
                )
                nc.sync.dma_start(out=dbg_k[:, :], in_=kT_sb)
                nc.sync.dma_start(
                    out=dbg_v.rearrange("p (n m) -> p n m", n=NTT), in_=vkd_sb
                )
                nc.sync.dma_start(out=dbg_y[:, :], in_=y_in[:, :])

            # ---------- phase 3: AllGather y.T across the 8 cores ----------
            nc.gpsimd.collective_compute(
                "AllGather",
                mybir.AluOpType.bypass,
                ins=[y_in[:, :]],
                outs=[y_all[:, :]],
                replica_groups=[list(range(N_CORES))],
            )

            # ---------- phase 4: out = y @ wo_c.T (column-parallel) ----------
            wo_sb = weights.tile([128, NKT, NREP * HD], F32, tag="wbig")
            nc.sync.dma_start(
                out=wo_sb, in_=woT.rearrange("(n p) m -> p n m", p=128)
            )
            y_r = y_all.rearrange("(n p) m -> p n m", p=128)
            for tt in range(NTT):
                # alternate between two dead slots for double buffering
                ytag = "qt" if tt % 2 == 0 else "kt"
                y_t = acts.tile([128, NKT, 128], F32, tag=ytag)
                nc.sync.dma_start(
                    out=y_t, in_=y_r[:, :, 128 * tt:128 * (tt + 1)]
                )
                o_ps = psum.tile([128, 512], F32, tag="bank")
                for kt in range(NKT):
                    nc.tensor.matmul(
                        o_ps,
                        lhsT=y_t[:, kt, :],
                        rhs=wo_sb[:, kt, :],
                        start=(kt == 0),
                        stop=(kt == NKT - 1),
                    )
                o_sb = outp.tile([128, 512], F32, tag="osb")
                nc.scalar.copy(o_sb, o_ps)
                nc.sync.dma_start(
                    out=out[128 * tt:128 * (tt + 1), :], in_=o_sb
                )

    nc.compile()
    _cached["nc"] = nc
    return nc


def _build_in_maps(inputs):
    return _shard_inputs(**inputs)


def _shard_inputs(x, cos, sin, wq, wk, wv, wo, start_pos):
    x = np.asarray(x, dtype=np.float32)
    cos = np.asarray(cos, dtype=np.float32)
    sin = np.asarray(sin, dtype=np.float32)
    wq = np.asarray(wq, dtype=np.float32)
    wk = np.asarray(wk, dtype=np.float32)
    wv = np.asarray(wv, dtype=np.float32)
    wo = np.asarray(wo, dtype=np.float32)
    sp = int(start_pos)

    xT = np.ascontiguousarray(x[0].T)  # (DIM, T)
    cosT = np.ascontiguousarray(cos[sp:sp + T].T)  # (64, T)
    sinT = np.ascontiguousarray(sin[sp:sp + T].T)
    cos2 = np.concatenate([cosT, cosT], axis=0)  # (128, T)
    sin2 = np.concatenate([-sinT, sinT], axis=0)  # rotate-half signs folded in

    kk = np.arange(128)[:, None]
    qq = np.arange(512)[None, :]
    masks = np.concatenate(
        [(kk + 128 * d <= qq).astype(np.float32) for d in range(4)], axis=1
    )  # (128, 2048)

    in_maps = []
    for c in range(N_CORES):
        qrows = slice(NREP * HD * c, NREP * HD * (c + 1))
        krows = slice(HD * c, HD * (c + 1))
        in_maps.append({
            "xT": xT,
            "cos2": cos2,
            "sin2": sin2,
            "masks": masks,
            "wqT": np.ascontiguousarray(wq[qrows, :].T),
            "wkT": np.ascontiguousarray(wk[krows, :].T),
            "wvT": np.ascontiguousarray(wv[krows, :].T),
            "woT": np.ascontiguousarray(wo[qrows, :].T),
        })
    return in_maps


def kernel(x, cos, sin, wq, wk, wv, wo, start_pos):
    in_maps = _shard_inputs(x, cos, sin, wq, wk, wv, wo, start_pos)
    nc = _build_kernel()
    res = run_bass_kernel_spmd(nc, in_maps, core_ids=list(range(N_CORES)))
    out = np.concatenate([res.results[c]["out"] for c in range(N_CORES)], axis=1)
    return out.reshape(1, T, DIM)



# revision 9
# speedup vs baseline: 3.8539x; 3.8539x over previous
"""GroupedQueryAttention (B=1, T=2048, D=4096, 32 q-heads / 8 kv-heads, hd=128)
on 8 trn2 NeuronCores.

Sharding: kv-head parallel — core c owns kv head c and its 4 query heads.
Mixed precision: fp16 for x/weights/q/k (projections, rope, score matmuls),
bf16 for exp/l/PV (bf16 has fp32 range — scaled scores reach ~50, exp(50)
overflows fp16), fp16 again for the normalized y and the column-parallel wo
matmul after a chunked AllGather. All matmuls run at 1 cycle/row (vs 4 for
fp32). Softmax without max-subtraction (bf16 exp can't overflow here).

Pipeline per 512-token chunk qc:
  QKV sweeps (x chunk in SBUF halves, one PSUM accumulator per sweep)
  -> rope -> causal attention in transposed [k, q] layout (exp fused over
  1024-wide score pairs, softmax denominator via ones-matmul, deferred
  per-strip epilogue so the PE never waits on the reciprocal)
  -> AllGather of the y.T chunk (overlapped with the next chunk's compute)
  -> column-parallel wo matmul for the previous chunk (its y reads are
  issued right after the next chunk's projections so the DMA is done by
  the time the PE gets there).
The last chunk's AllGather is split by head pairs so it overlaps the last
two attention strips; wo for that chunk indexes wo_sb tiles to match the
rank-major order of the two half-gathers.
"""
import sys

sys.path.insert(0, "/opt/trn_rl_repo")

import numpy as np
import ml_dtypes

import concourse.bacc as bacc
import concourse.tile as tile
from concourse import mybir
from concourse.bass_utils import run_bass_kernel_spmd
from concourse.masks import make_identity

N_CORES = 8
T = 2048
DIM = 4096
HD = 128
NH = 32
NKV = 8
NREP = NH // NKV  # 4 query heads per core
CH = 512  # chunk length along T
NCHUNK = T // CH  # 4
NKT = DIM // 128  # 32 contraction tiles for the projections
F32 = mybir.dt.float32
F16 = mybir.dt.float16
BF16 = mybir.dt.bfloat16
SCALE = 1.0 / float(np.sqrt(HD))
BF16NP = ml_dtypes.bfloat16
LAST = NCHUNK - 1

_cached = {}


def _build_kernel():
    if "nc" in _cached:
        return _cached["nc"]

    nc = bacc.Bacc("TRN2", target_bir_lowering=False)

    xt = nc.dram_tensor("xt", [NCHUNK, DIM, CH], F16, kind="ExternalInput")
    cos2 = nc.dram_tensor("cos2", [128, T], F16, kind="ExternalInput")
    sin2 = nc.dram_tensor("sin2", [128, T], F16, kind="ExternalInput")
    masks = nc.dram_tensor("masks", [128, 4 * CH], BF16, kind="ExternalInput")
    # weight groups pre-transposed on host to [128, NKT*128] per group,
    # in sweep order v, q0..q3, k — each group load is one contiguous DMA
    wqkv = nc.dram_tensor("wqkv", [6, 128, NKT * 128], F16, kind="ExternalInput")
    woT = nc.dram_tensor("woT", [DIM, NREP * HD], F16, kind="ExternalInput")
    out = nc.dram_tensor("out", [T, NREP * HD], F16, kind="ExternalOutput")

    y_in = [
        nc.dram_tensor(f"y_in{qc}", [NREP * HD, CH], F16, kind="Internal")
        for qc in range(LAST)
    ]
    y_all = [
        nc.dram_tensor(
            f"y_all{qc}", [DIM, CH], F16, kind="Internal", addr_space="Shared"
        )
        for qc in range(LAST)
    ]
    # last chunk: two half-gathers (head pairs) so the collective overlaps
    # the tail attention strips
    y_in_l = [
        nc.dram_tensor(f"y_inl{i}", [2 * HD, CH], F16, kind="Internal")
        for i in range(2)
    ]
    y_all_l = [
        nc.dram_tensor(
            f"y_alll{i}", [DIM // 2, CH], F16, kind="Internal",
            addr_space="Shared",
        )
        for i in range(2)
    ]

    with tile.TileContext(nc) as tc:
        with (
            tc.tile_pool(name="consts", bufs=1) as consts,
            tc.tile_pool(name="weights", bufs=1) as weights,
            tc.tile_pool(name="xpool", bufs=3) as xpool,
            tc.tile_pool(name="acts", bufs=1) as acts,
            tc.tile_pool(name="qpool", bufs=1) as qpool,
            tc.tile_pool(name="work", bufs=2) as work,
            tc.tile_pool(name="expp", bufs=3) as expp,
            tc.tile_pool(name="outp", bufs=2) as outp,
            tc.tile_pool(name="ypool", bufs=3) as ypool,
            tc.tile_pool(name="pp", bufs=4, space="PSUM") as pp,
            tc.tile_pool(name="sp", bufs=2, space="PSUM") as sp,
        ):
            def emit_x_load(qc):
                xh = []
                for i in range(2):
                    x_sb = xpool.tile(
                        [128, NKT // 2, CH], F16, tag="xsb", name=f"x{qc}_{i}"
                    )
                    nc.sync.dma_start(
                        out=x_sb,
                        in_=xt[qc, 2048 * i:2048 * (i + 1), :].rearrange(
                            "(n p) m -> p n m", p=128
                        ),
                    )
                    xh.append(x_sb)
                return xh

            # ---------- startup loads, in first-consumption order ----------
            wqkv_sb = weights.tile([128, 6, NKT, 128], F16, tag="wqkv")
            wq_r = wqkv.rearrange("g p (n m) -> g p n m", m=128)
            nc.sync.dma_start(out=wqkv_sb[:, 0], in_=wq_r[0])  # v group
            x_first = emit_x_load(0)
            for g in range(1, 6):  # q0..q3, k groups
                nc.sync.dma_start(out=wqkv_sb[:, g], in_=wq_r[g])

            cos_sb = consts.tile([128, T], F16, tag="cos")
            nc.sync.dma_start(out=cos_sb, in_=cos2[:, :])
            sin_sb = consts.tile([128, T], F16, tag="sin")
            nc.sync.dma_start(out=sin_sb, in_=sin2[:, :])
            mask_sb = consts.tile([128, 4 * CH], BF16, tag="mask")
            nc.sync.dma_start(out=mask_sb, in_=masks[:, :])
            ones_col = consts.tile([128, 1], BF16, tag="onesc")
            nc.vector.memset(ones_col, 1.0)
            ones_row = consts.tile([1, 128], BF16, tag="onesr")
            nc.vector.memset(ones_row, 1.0)
            ident = consts.tile([128, 128], BF16, tag="ident")
            make_identity(nc, ident)

            wo_sb = weights.tile([128, NKT, NREP * HD], F16, tag="wo")
            nc.sync.dma_start(
                out=wo_sb, in_=woT.rearrange("(n p) m -> p n m", p=128)
            )

            # K^T and V tiles persist across the whole attention phase
            kT_sb = acts.tile([128, T], F16, tag="kt")
            vkd_sb = acts.tile([128, T // 128, HD], BF16, tag="vkd")

            def qkv_phase(qc, xh):
                cs = slice(CH * qc, CH * (qc + 1))

                def xtile(kt):
                    return xh[kt // 16][:, kt % 16, :]

                # sweep order: v (group 0) first so the transposes at the
                # end have their input, then q0..q3, k (groups 1..5)
                v_ps = pp.tile([128, CH], F32, tag="bank", name=f"vps{qc}")
                for kt in range(NKT):
                    nc.tensor.matmul(
                        v_ps,
                        lhsT=wqkv_sb[:, 0, kt, :],
                        rhs=xtile(kt),
                        start=(kt == 0),
                        stop=(kt == NKT - 1),
                    )
                v_sb = work.tile([128, CH], BF16, tag="vsb")
                nc.scalar.copy(v_sb, v_ps)

                qT_sb = qpool.tile([128, NREP, CH], F16, tag="qt")
                for h in range(NREP + 1):
                    a_ps = pp.tile([128, CH], F32, tag="bank", name=f"aps{qc}_{h}")
                    for kt in range(NKT):
                        nc.tensor.matmul(
                            a_ps,
                            lhsT=wqkv_sb[:, 1 + h, kt, :],
                            rhs=xtile(kt),
                            start=(kt == 0),
                            stop=(kt == NKT - 1),
                        )
                    psb = work.tile([128, CH], F16, tag="psb")
                    nc.scalar.copy(psb, a_ps)
                    tmp = work.tile([128, CH], F16, tag="tmp")
                    nc.scalar.copy(tmp[0:64, :], a_ps[64:128, :])
                    nc.scalar.copy(tmp[64:128, :], a_ps[0:64, :])
                    dst = qT_sb[:, h, :] if h < NREP else kT_sb[:, cs]
                    nc.vector.tensor_mul(dst, psb, cos_sb[:, cs])
                    nc.vector.tensor_mul(tmp, tmp, sin_sb[:, cs])
                    nc.vector.tensor_add(dst, dst, tmp)

                # v computed in [hd, T] layout; transpose 128x128 blocks to [k, hd]
                for s in range(4):
                    vt_ps = pp.tile([128, 128], BF16, tag="bank", name=f"vt{qc}_{s}")
                    nc.tensor.transpose(vt_ps, v_sb[:, 128 * s:128 * (s + 1)], ident)
                    nc.vector.tensor_copy(vkd_sb[:, 4 * qc + s, :], vt_ps)
                return qT_sb

            def ag(ins_t, outs_t):
                nc.gpsimd.collective_compute(
                    "AllGather",
                    mybir.AluOpType.bypass,
                    ins=[ins_t[:, :]],
                    outs=[outs_t[:, :]],
                    replica_groups=[list(range(N_CORES))],
                )

            def att_phase(qc, qT_sb):
                nkt = 4 * qc + 4  # causal: k tiles 0 .. 4*qc+3
                pairs = nkt // 2
                last = qc == LAST
                prev_epi = None
                for h in range(NREP):
                    q_rhs = qT_sb[:, h, :]
                    yT_ps = pp.tile([128, CH], F32, tag="bank", name=f"yT{qc}_{h}")
                    l_acc = work.tile([128, 1024], BF16, tag="lacc")
                    es = []
                    for j in range(pairs):
                        s_ps = sp.tile([128, 1024], F32, tag="pair")
                        for d2 in range(2):
                            kt = 2 * j + d2
                            nc.tensor.matmul(
                                s_ps[:, 512 * d2:512 * (d2 + 1)],
                                lhsT=kT_sb[:, 128 * kt:128 * (kt + 1)],
                                rhs=q_rhs,
                                start=True,
                                stop=True,
                            )
                        e_sb = expp.tile([128, 1024], BF16, tag="exp")
                        nc.scalar.activation(
                            e_sb, s_ps, mybir.ActivationFunctionType.Exp,
                            scale=SCALE,
                        )
                        if j >= pairs - 2:  # diagonal pair: zero the k > q half
                            dd = j - (pairs - 2)
                            nc.vector.tensor_mul(
                                e_sb, e_sb, mask_sb[:, 1024 * dd:1024 * (dd + 1)]
                            )
                        if j == 0:
                            nc.vector.tensor_copy(l_acc, e_sb)
                        else:
                            nc.vector.tensor_add(l_acc, l_acc, e_sb)
                        es.append(e_sb)
                        if j == 1 and prev_epi is not None:
                            prev_epi()
                            prev_epi = None
                            if last and h == 2:
                                ag(y_in_l[0], y_all_l[0])  # heads 0,1 gathered
                        if j >= 1:  # PV for the previous pair (keeps PE fed)
                            pj = j - 1
                            for d2 in range(2):
                                kt = 2 * pj + d2
                                nc.tensor.matmul(
                                    yT_ps,
                                    lhsT=vkd_sb[:, kt, :],
                                    rhs=es[pj][:, 512 * d2:512 * (d2 + 1)],
                                    start=(kt == 0),
                                    stop=False,
                                )
                    for d2 in range(2):  # PV for the last pair
                        kt = 2 * (pairs - 1) + d2
                        nc.tensor.matmul(
                            yT_ps,
                            lhsT=vkd_sb[:, kt, :],
                            rhs=es[pairs - 1][:, 512 * d2:512 * (d2 + 1)],
                            start=(kt == 0),
                            stop=(kt == nkt - 1),
                        )
                    # softmax denominator: sum l_acc halves over partitions
                    l_ps = pp.tile([1, CH], F32, tag="bank", name=f"l{qc}_{h}")
                    nc.tensor.matmul(
                        l_ps[0:1, :], lhsT=ones_col[:, 0:1],
                        rhs=l_acc[:, 0:512], start=True, stop=False,
                    )
                    nc.tensor.matmul(
                        l_ps[0:1, :], lhsT=ones_col[:, 0:1],
                        rhs=l_acc[:, 512:1024], start=False, stop=True,
                    )
                    r32 = work.tile([1, CH], F32, tag="r32")
                    nc.vector.reciprocal_approx_fast(r32, l_ps[0:1, :])
                    r16 = work.tile([1, CH], BF16, tag="r16")
                    nc.vector.tensor_copy(r16, r32)

                    def epi(yT_ps=yT_ps, r16=r16, h=h, qc=qc, last=last):
                        bc_ps = pp.tile([128, CH], F32, tag="bank")
                        nc.tensor.matmul(
                            bc_ps, lhsT=ones_row[0:1, :], rhs=r16[0:1, :],
                            start=True, stop=True,
                        )
                        bc_sb = work.tile([128, CH], F32, tag="bc")
                        nc.scalar.copy(bc_sb, bc_ps)
                        yn_sb = work.tile([128, CH], F16, tag="yn")
                        nc.vector.tensor_mul(yn_sb, yT_ps, bc_sb)
                        if last:
                            dst = y_in_l[h // 2][128 * (h % 2):128 * (h % 2 + 1), :]
                        else:
                            dst = y_in[qc][128 * h:128 * (h + 1), :]
                        nc.sync.dma_start(out=dst, in_=yn_sb)

                    prev_epi = epi
                prev_epi()  # y_in[qc] must be complete before its AllGather
                if last:
                    ag(y_in_l[1], y_all_l[1])
                else:
                    ag(y_in[qc], y_all[qc])

            def wo_start(qc):
                # early y reads for the first two row-tiles: issued on the
                # sync queue ahead of the attention phase's writes
                y_r = y_all[qc].rearrange("(n p) m -> p n m", p=128)
                pre = []
                for tt in range(2):
                    y_t = ypool.tile([128, NKT, 128], F16, tag="yt")
                    nc.sync.dma_start(
                        out=y_t, in_=y_r[:, :, 128 * tt:128 * (tt + 1)]
                    )
                    pre.append(y_t)
                return pre

            def wo_compute(qc, pre):
                y_r = y_all[qc].rearrange("(n p) m -> p n m", p=128)
                for tt in range(CH // 128):
                    if tt < len(pre):
                        y_t = pre[tt]
                    else:
                        y_t = ypool.tile([128, NKT, 128], F16, tag="yt")
                        nc.sync.dma_start(
                            out=y_t, in_=y_r[:, :, 128 * tt:128 * (tt + 1)]
                        )
                    o_ps = pp.tile([128, NREP * HD], F32, tag="bank")
                    for kt in range(NKT):
                        nc.tensor.matmul(
                            o_ps,
                            lhsT=y_t[:, kt, :],
                            rhs=wo_sb[:, kt, :],
                            start=(kt == 0),
                            stop=(kt == NKT - 1),
                        )
                    o_sb = outp.tile([128, NREP * HD], F16, tag="osb")
                    nc.scalar.copy(o_sb, o_ps)
                    r0 = CH * qc + 128 * tt
                    nc.sync.dma_start(out=out[r0:r0 + 128, :], in_=o_sb)

            def wo_last():
                # chunk LAST: y arrives as two half-gathers, rank-major with
                # 2 head-tiles per rank; map each to its wo_sb k-tile
                y_rs = [
                    y_all_l[i].rearrange("(n p) m -> p n m", p=128)
                    for i in range(2)
                ]
                for tt in range(CH // 128):
                    y_ts = []
                    for i in range(2):
                        y_t = ypool.tile([128, NKT // 2, 128], F16, tag="yt")
                        nc.sync.dma_start(
                            out=y_t, in_=y_rs[i][:, :, 128 * tt:128 * (tt + 1)]
                        )
                        y_ts.append(y_t)
                    o_ps = pp.tile([128, NREP * HD], F32, tag="bank")
                    for i in range(2):
                        for n in range(NKT // 2):
                            kt = 4 * (n // 2) + 2 * i + (n % 2)
                            nc.tensor.matmul(
                                o_ps,
                                lhsT=y_ts[i][:, n, :],
                                rhs=wo_sb[:, kt, :],
                                start=(i == 0 and n == 0),
                                stop=(i == 1 and n == NKT // 2 - 1),
                            )
                    o_sb = outp.tile([128, NREP * HD], F16, tag="osb")
                    nc.scalar.copy(o_sb, o_ps)
                    r0 = CH * LAST + 128 * tt
                    nc.sync.dma_start(out=out[r0:r0 + 128, :], in_=o_sb)

            xh = x_first
            for qc in range(NCHUNK):
                xh_next = emit_x_load(qc + 1) if qc + 1 < NCHUNK else None
                qT_sb = qkv_phase(qc, xh)
                xh = xh_next
                pre = wo_start(qc - 1) if qc >= 1 else None
                att_phase(qc, qT_sb)
                if qc >= 1:
                    wo_compute(qc - 1, pre)
            wo_last()

    nc.compile()
    _cached["nc"] = nc
    return nc


def _build_in_maps(inputs):
    return _shard_inputs(**inputs)


def _shard_inputs(x, cos, sin, wq, wk, wv, wo, start_pos):
    x = np.asarray(x, dtype=np.float32)
    cos = np.asarray(cos, dtype=np.float32)
    sin = np.asarray(sin, dtype=np.float32)
    wq = np.asarray(wq, dtype=np.float32)
    wk = np.asarray(wk, dtype=np.float32)
    wv = np.asarray(wv, dtype=np.float32)
    wo = np.asarray(wo, dtype=np.float32)
    sp = int(start_pos)

    xT = x[0].T  # (DIM, T)
    xt = np.ascontiguousarray(
        xT.reshape(DIM, NCHUNK, CH).transpose(1, 0, 2)
    ).astype(np.float16)  # (NCHUNK, DIM, CH)
    cosT = cos[sp:sp + T].T  # (64, T)
    sinT = sin[sp:sp + T].T
    cos2 = np.concatenate([cosT, cosT], axis=0).astype(np.float16)  # (128, T)
    sin2 = np.concatenate([-sinT, sinT], axis=0).astype(np.float16)

    kk = np.arange(128)[:, None]
    qq = np.arange(CH)[None, :]
    masks = np.concatenate(
        [(kk + 128 * d <= qq).astype(np.float32) for d in range(4)], axis=1
    ).astype(BF16NP)  # (128, 2048)

    def wgroup(wmat):  # (DIM, 128) -> (128, NKT*128), partition-major
        return wmat.reshape(NKT, 128, 128).transpose(1, 0, 2).reshape(
            128, NKT * 128
        )

    in_maps = []
    for c in range(N_CORES):
        qrows = slice(NREP * HD * c, NREP * HD * (c + 1))
        krows = slice(HD * c, HD * (c + 1))
        wqc = wq[qrows, :].T  # (DIM, 512)
        groups = [wv[krows, :].T] + [
            wqc[:, 128 * h:128 * (h + 1)] for h in range(NREP)
        ] + [wk[krows, :].T]
        wqkv = np.stack([wgroup(g) for g in groups]).astype(np.float16)
        in_maps.append({
            "xt": xt,
            "cos2": cos2,
            "sin2": sin2,
            "masks": masks,
            "wqkv": np.ascontiguousarray(wqkv),
            "woT": np.ascontiguousarray(wo[qrows, :].T).astype(np.float16),
        })
    return in_maps


def kernel(x, cos, sin, wq, wk, wv, wo, start_pos):
    in_maps = _shard_inputs(x, cos, sin, wq, wk, wv, wo, start_pos)
    nc = _build_kernel()
    res = run_bass_kernel_spmd(nc, in_maps, core_ids=list(range(N_CORES)))
    out = np.concatenate(
        [res.results[c]["out"].astype(np.float32) for c in range(N_CORES)],
        axis=1,
    )
    return out.reshape(1, T, DIM)
